# revision 5
# baseline (speedup 1.0000x reference)
"""Trainium2 Bass kernel for nn_FC_72232759984564 (GNN message passing + MLP).

Strategy (8 NeuronCores, SPMD, batch-parallel):
  * The final [d_index]/[p_index] gathers make all but ~4k of the 50k/20k
    GCN output rows dead.  For each core's 512-row batch shard the host
    builds, per GCN branch, the set of needed node rows (gather targets +
    their in-edge sources), a gathered node-feature matrix Xg [R, F], and
    a dense coefficient matrix C^T [R, 512] encoding the symmetric-norm
    weighted adjacency (+ self loops), so that the branch output is
    leaky(W^T @ (Xg^T @ C^T) + b)  — two dense matmuls per branch.
  * All activations live transposed on-chip: [features(partitions), batch].
    BatchNorm (training mode, batch axis) needs global batch stats: each
    core computes per-feature sum/sumsq and a small AllReduce (~KB) merges
    them; a dummy AllReduce issued at kernel start absorbs the first-
    collective setup cost while the PE array is busy with the GCN branches.
  * Matmuls run in float32r (full fp32 data, ~2e-4 matmul rounding, full
    PE rate).  Feature concat boundaries are padded to multiples of 128 on
    the host (W_e1 rows are permuted to match), so every tile is dense.
"""

import numpy as np

import concourse.bacc as bacc
import concourse.mybir as mybir
import concourse.tile as tile
from concourse.bass_utils import run_bass_kernel_spmd

N_CORES = 8
B = 4096
BL = B // N_CORES            # 512 batch rows per core

ND, NP = 50000, 20000        # node counts
DX = 1024                    # d_ecfps feature dim (already 128-aligned)
PX, PX_PAD = 2812, 2816      # p_gos feature dim, padded
DV, PEM = 300, 1024          # d_vecs / p_embeddings dims
FIN_ROWS = 1408              # 384 (dv padded to 3 tiles) + 1024 (pe)
H1, H2, H3 = 2048, 1024, 256
FDIM_PAD = 3456              # 384 + 1024 + 1024 + 1024 = 27 * 128
RD_MAX, RP_MAX = 1536, 2560  # padded gathered-row counts (>=8 sigma margin)
EPS = 1e-5

F32 = mybir.dt.float32
F32R = mybir.dt.float32r
AF = mybir.ActivationFunctionType
AX = mybir.AxisListType
RG = [list(range(N_CORES))]

KD = RD_MAX // 128           # 12
KP = RP_MAX // 128           # 20
MD = DX // 128               # 8   (G_d row tiles)
MP = PX_PAD // 128           # 22  (G_p row tiles)
MH = H2 // 128               # 8   (branch output tiles)
KF = FDIM_PAD // 128         # 27
M1 = H1 // 128               # 16
M2 = H2 // 128               # 8
M3 = H3 // 128               # 2

_PROGRAMS = {}


def _col_block(ap, m, kt):
    """DRAM AP for a [kt*128, 128] column block m, tiled [128, kt, 128]."""
    return ap[:, m * 128:(m + 1) * 128].rearrange("(k p) c -> p k c", p=128)


def _build_program(rd_max, rp_max):
    kd, kp = rd_max // 128, rp_max // 128
    nc = bacc.Bacc("TRN2", target_bir_lowering=False, debug=False,
                   num_devices=N_CORES)

    def din(name, shape, dt=F32R):
        return nc.dram_tensor(name, shape, dt, kind="ExternalInput").ap()

    xd = din("xd", [rd_max, DX])
    cdt = din("cdt", [rd_max, BL])
    xp = din("xp", [rp_max, PX_PAD])
    cpt = din("cpt", [rp_max, BL])
    fin = din("fin", [FIN_ROWS, BL])
    w_dg = din("w_dg", [DX, H2])
    w_pg = din("w_pg", [PX_PAD, H2])
    w_e1 = din("w_e1", [FDIM_PAD, H1])
    w_e2 = din("w_e2", [H1, H2])
    w_o1 = din("w_o1", [H2, H3])
    w_o2 = din("w_o2", [H3, 1])
    b_dg = din("b_dg", [H2], F32)
    b_pg = din("b_pg", [H2], F32)
    g_e1 = din("g_e1", [H1], F32)
    be_e1 = din("be_e1", [H1], F32)
    g_e2 = din("g_e2", [H2], F32)
    be_e2 = din("be_e2", [H2], F32)
    b_o1 = din("b_o1", [H3], F32)
    g_o = din("g_o", [H3], F32)
    be_o = din("be_o", [H3], F32)
    b_o2 = din("b_o2", [1], F32)

    ft_out = nc.dram_tensor("ft_out", [H2, BL], F32R, kind="ExternalOutput").ap()
    yt_out = nc.dram_tensor("yt_out", [1, BL], F32, kind="ExternalOutput").ap()
    dbg_out = nc.dram_tensor("dbg_out", [128, 1], F32, kind="ExternalOutput").ap()

    with tile.TileContext(nc) as tc:
        with (
            tc.tile_pool(name="const", bufs=1) as cp,
            tc.tile_pool(name="ws", bufs=2) as ws,
            tc.tile_pool(name="psum", bufs=4, space="PSUM") as pp,
            tc.tile_pool(name="dram", bufs=1, space="DRAM") as dp,
            tc.tile_pool(name="ft", bufs=1) as ftp,
        ):
            # ---- dummy collective: absorb first-collective setup early ----
            zt = cp.tile([128, 1], F32, tag="zt")
            nc.vector.memset(zt[:], 0.0)
            ar0_in = dp.tile([128, 1], F32, tag="ar0i")
            ar0_out = dp.tile([128, 1], F32, tag="ar0o")
            nc.sync.dma_start(ar0_in[:], zt[:])
            nc.gpsimd.collective_compute(
                "AllReduce", mybir.AluOpType.add, replica_groups=RG,
                ins=[ar0_in.opt()], outs=[ar0_out.opt()])
            nc.sync.dma_start(dbg_out, ar0_out[:])

            ft_t = ftp.tile([128, KF, BL], F32R, tag="ft")
            nc.sync.dma_start(ft_t[:, 0:11, :],
                              fin.rearrange("(t p) b -> p t b", p=128))

            def vec_tile(src, nm, name):
                t = cp.tile([128, nm], F32, tag=name)
                nc.sync.dma_start(t[:], src.rearrange("(m p) -> p m", p=128))
                return t

            b_dg_t = vec_tile(b_dg, MH, "b_dg")
            b_pg_t = vec_tile(b_pg, MH, "b_pg")
            g_e1_t = vec_tile(g_e1, M1, "g_e1")
            be_e1_t = vec_tile(be_e1, M1, "be_e1")
            g_e2_t = vec_tile(g_e2, M2, "g_e2")
            be_e2_t = vec_tile(be_e2, M2, "be_e2")
            b_o1_t = vec_tile(b_o1, M3, "b_o1")
            g_o_t = vec_tile(g_o, M3, "g_o")
            be_o_t = vec_tile(be_o, M3, "be_o")
            b_o2_t = cp.tile([1, 1], F32, tag="b_o2")
            nc.sync.dma_start(b_o2_t[:], b_o2.unsqueeze(1))

            sq = cp.tile([128, BL], F32, tag="sq")

            # ================= branch D =================
            with tc.tile_pool(name="brd", bufs=1) as bd:
                cdt_t = bd.tile([128, kd, BL], F32R, tag="cdt")
                nc.sync.dma_start(cdt_t[:], cdt.rearrange("(k p) b -> p k b", p=128))
                gd_t = bd.tile([128, MD, BL], F32R, tag="gd")
                for m in range(MD):
                    xdb = ws.tile([128, kd, 128], F32R, tag="wblk")
                    nc.sync.dma_start(xdb[:], _col_block(xd, m, kd))
                    ps = pp.tile([128, BL], F32, tag="mm")
                    for k in range(kd):
                        nc.tensor.matmul(ps[:], xdb[:, k, :], cdt_t[:, k, :],
                                         start=(k == 0), stop=(k == kd - 1))
                    nc.vector.tensor_copy(gd_t[:, m, :], ps[:])
                for m in range(MH):
                    wdb = ws.tile([128, MD, 128], F32R, tag="wblk")
                    nc.sync.dma_start(wdb[:], _col_block(w_dg, m, MD))
                    ps = pp.tile([128, BL], F32, tag="mm")
                    for k in range(MD):
                        nc.tensor.matmul(ps[:], wdb[:, k, :], gd_t[:, k, :],
                                         start=(k == 0), stop=(k == MD - 1))
                    nc.scalar.activation(ft_t[:, 11 + m, :], ps[:], AF.Lrelu,
                                         bias=b_dg_t[:, m:m + 1], alpha=0.01)

            # ================= branch P =================
            with tc.tile_pool(name="brp", bufs=1) as bp:
                cpt_t = bp.tile([128, kp, BL], F32R, tag="cpt")
                nc.sync.dma_start(cpt_t[:], cpt.rearrange("(k p) b -> p k b", p=128))
                gp_t = bp.tile([128, MP, BL], F32R, tag="gp")
                for m in range(MP):
                    xpb = ws.tile([128, kp, 128], F32R, tag="wblk")
                    nc.sync.dma_start(xpb[:], _col_block(xp, m, kp))
                    ps = pp.tile([128, BL], F32, tag="mm")
                    for k in range(kp):
                        nc.tensor.matmul(ps[:], xpb[:, k, :], cpt_t[:, k, :],
                                         start=(k == 0), stop=(k == kp - 1))
                    nc.vector.tensor_copy(gp_t[:, m, :], ps[:])
                for m in range(MH):
                    wpb = ws.tile([128, MP, 128], F32R, tag="wblk")
                    nc.sync.dma_start(wpb[:], _col_block(w_pg, m, MP))
                    ps = pp.tile([128, BL], F32, tag="mm")
                    for k in range(MP):
                        nc.tensor.matmul(ps[:], wpb[:, k, :], gp_t[:, k, :],
                                         start=(k == 0), stop=(k == MP - 1))
                    nc.scalar.activation(ft_t[:, 19 + m, :], ps[:], AF.Lrelu,
                                         bias=b_pg_t[:, m:m + 1], alpha=0.01)

            # ================= MLP =================
            def bn_scale_shift(stats_g, nm, g_t, be_t, name):
                """stats_g [128, 2*nm] (sums in 0:nm, sumsqs in nm:2nm) ->
                per-feature scale/shift [128, nm]."""
                mean = cp.tile([128, nm], F32, tag=f"{name}_mean")
                ex2 = cp.tile([128, nm], F32, tag=f"{name}_ex2")
                var = cp.tile([128, nm], F32, tag=f"{name}_var")
                std = cp.tile([128, nm], F32, tag=f"{name}_std")
                scl = cp.tile([128, nm], F32, tag=f"{name}_scl")
                shf = cp.tile([128, nm], F32, tag=f"{name}_shf")
                nc.vector.tensor_scalar_mul(mean[:], stats_g[:, 0:nm], 1.0 / B)
                nc.vector.tensor_scalar_mul(ex2[:], stats_g[:, nm:2 * nm], 1.0 / B)
                nc.vector.tensor_mul(var[:], mean[:], mean[:])
                nc.vector.tensor_sub(var[:], ex2[:], var[:])
                nc.vector.tensor_scalar_add(var[:], var[:], EPS)
                nc.scalar.activation(std[:], var[:], AF.Sqrt)
                nc.vector.reciprocal(std[:], std[:])
                nc.vector.tensor_mul(scl[:], std[:], g_t[:])
                nc.vector.tensor_mul(shf[:], mean[:], scl[:])
                nc.vector.tensor_sub(shf[:], be_t[:], shf[:])
                return scl, shf

            def all_reduce_stats(stats, nm, name):
                ar_in = dp.tile([128, 2 * nm], F32, tag=f"{name}_i")
                ar_out = dp.tile([128, 2 * nm], F32, tag=f"{name}_o")
                nc.sync.dma_start(ar_in[:], stats[:])
                nc.gpsimd.collective_compute(
                    "AllReduce", mybir.AluOpType.add, replica_groups=RG,
                    ins=[ar_in.opt()], outs=[ar_out.opt()])
                stats_g = cp.tile([128, 2 * nm], F32, tag=f"{name}_g")
                nc.sync.dma_start(stats_g[:], ar_out[:])
                return stats_g

            with tc.tile_pool(name="mlp", bufs=1) as mp:
                # ---- layer e1 ----
                stats1 = cp.tile([128, 2 * M1], F32, tag="stats1")
                z1_t = mp.tile([128, M1, BL], F32R, tag="z1")
                for m in range(M1):
                    wb = ws.tile([128, KF, 128], F32R, tag="wblk")
                    nc.sync.dma_start(wb[:], _col_block(w_e1, m, KF))
                    ps = pp.tile([128, BL], F32, tag="mm")
                    for k in range(KF):
                        nc.tensor.matmul(ps[:], wb[:, k, :], ft_t[:, k, :],
                                         start=(k == 0), stop=(k == KF - 1))
                    nc.vector.tensor_copy(z1_t[:, m, :], ps[:])
                    nc.vector.reduce_sum(stats1[:, m:m + 1], ps[:], axis=AX.X)
                    nc.scalar.activation(sq[:], ps[:], AF.Square,
                                         accum_out=stats1[:, M1 + m:M1 + m + 1])
                stats1_g = all_reduce_stats(stats1, M1, "ar1")
                scl1, shf1 = bn_scale_shift(stats1_g, M1, g_e1_t, be_e1_t, "bn1")
                for m in range(M1):
                    nc.scalar.activation(z1_t[:, m, :], z1_t[:, m, :], AF.Lrelu,
                                         bias=shf1[:, m:m + 1],
                                         scale=scl1[:, m:m + 1], alpha=0.01)

                # ---- layer e2 ----
                stats2 = cp.tile([128, 2 * M2], F32, tag="stats2")
                z2_t = mp.tile([128, M2, BL], F32R, tag="z2")
                f_t = mp.tile([128, M2, BL], F32R, tag="f")
                for m in range(M2):
                    wb = ws.tile([128, M1, 128], F32R, tag="wblk")
                    nc.sync.dma_start(wb[:], _col_block(w_e2, m, M1))
                    ps = pp.tile([128, BL], F32, tag="mm")
                    for k in range(M1):
                        nc.tensor.matmul(ps[:], wb[:, k, :], z1_t[:, k, :],
                                         start=(k == 0), stop=(k == M1 - 1))
                    nc.vector.tensor_copy(z2_t[:, m, :], ps[:])
                    nc.vector.reduce_sum(stats2[:, m:m + 1], ps[:], axis=AX.X)
                    nc.scalar.activation(sq[:], ps[:], AF.Square,
                                         accum_out=stats2[:, M2 + m:M2 + m + 1])
                stats2_g = all_reduce_stats(stats2, M2, "ar2")
                scl2, shf2 = bn_scale_shift(stats2_g, M2, g_e2_t, be_e2_t, "bn2")
                for m in range(M2):
                    nc.scalar.activation(f_t[:, m, :], z2_t[:, m, :], AF.Lrelu,
                                         bias=shf2[:, m:m + 1],
                                         scale=scl2[:, m:m + 1], alpha=0.01)
                nc.sync.dma_start(ft_out.rearrange("(m p) b -> p m b", p=128),
                                  f_t[:])

                # ---- output head ----
                w_o1_t = cp.tile([128, M2, H3], F32R, tag="w_o1")
                nc.sync.dma_start(w_o1_t[:],
                                  w_o1.rearrange("(k p) c -> p k c", p=128))
                stats3 = cp.tile([128, 2 * M3], F32, tag="stats3")
                ot_t = cp.tile([128, M3, BL], F32, tag="ot")
                for m in range(M3):
                    ps = pp.tile([128, BL], F32, tag="mm")
                    for k in range(M2):
                        nc.tensor.matmul(ps[:], w_o1_t[:, k, m * 128:(m + 1) * 128],
                                         f_t[:, k, :],
                                         start=(k == 0), stop=(k == M2 - 1))
                    nc.scalar.activation(ot_t[:, m, :], ps[:], AF.Lrelu,
                                         bias=b_o1_t[:, m:m + 1], alpha=0.01)
                    nc.vector.reduce_sum(stats3[:, m:m + 1], ot_t[:, m, :], axis=AX.X)
                    nc.scalar.activation(sq[:], ot_t[:, m, :], AF.Square,
                                         accum_out=stats3[:, M3 + m:M3 + m + 1])
                stats3_g = all_reduce_stats(stats3, M3, "ar3")
                scl3, shf3 = bn_scale_shift(stats3_g, M3, g_o_t, be_o_t, "bn3")
                on_t = cp.tile([128, M3, BL], F32R, tag="on")
                for m in range(M3):
                    nc.scalar.activation(on_t[:, m, :], ot_t[:, m, :], AF.Identity,
                                         bias=shf3[:, m:m + 1],
                                         scale=scl3[:, m:m + 1])
                w_o2_t = cp.tile([128, M3, 1], F32R, tag="w_o2")
                nc.sync.dma_start(w_o2_t[:],
                                  w_o2.rearrange("(k p) o -> p k o", p=128))
                ps_y = pp.tile([128, BL], F32, tag="mm")
                for k in range(M3):
                    nc.tensor.matmul(ps_y[0:1, :], w_o2_t[:, k, :], on_t[:, k, :],
                                     start=(k == 0), stop=(k == M3 - 1))
                yt_t = cp.tile([1, BL], F32, tag="yt")
                nc.scalar.activation(yt_t[:], ps_y[0:1, :], AF.Identity,
                                     bias=b_o2_t[0:1, 0:1])
                nc.sync.dma_start(yt_out, yt_t[:])

    nc.compile()
    return nc


def _get_program(rd_max, rp_max):
    key = (rd_max, rp_max)
    if key not in _PROGRAMS:
        _PROGRAMS[key] = _build_program(rd_max, rp_max)
    return _PROGRAMS[key]


def _prep_branch(index_full, x_nodes, edge_index, edge_w, n_nodes, r_max, k_pad):
    """Per-core gathered node features + transposed coefficient matrices."""
    src = np.asarray(edge_index[0], dtype=np.int64)
    dst = np.asarray(edge_index[1], dtype=np.int64)
    w = np.asarray(edge_w, dtype=np.float32)
    index_full = np.asarray(index_full, dtype=np.int64)
    x_nodes = np.asarray(x_nodes, dtype=np.float32)

    deg = (np.bincount(dst, weights=w.astype(np.float64), minlength=n_nodes)
           .astype(np.float32) + np.float32(1.0))
    dis = np.float32(1.0) / np.sqrt(deg)
    norm = dis[src] * w * dis[dst]

    xgs, cts, rmax_seen = [], [], 0
    for c in range(N_CORES):
        idx = index_full[c * BL:(c + 1) * BL]
        uniq, inv = np.unique(idx, return_inverse=True)
        sel = np.isin(dst, uniq)
        es, ed, en = src[sel], dst[sel], norm[sel]
        cols = np.union1d(es, uniq)
        r = len(cols)
        rmax_seen = max(rmax_seen, r)
        if r > r_max:
            xgs.append(None)
            continue
        cnt = np.zeros((r_max, len(uniq)), np.float32)
        np.add.at(cnt, (np.searchsorted(cols, es), np.searchsorted(uniq, ed)), en)
        cnt[np.searchsorted(cols, uniq), np.arange(len(uniq))] += dis[uniq] ** 2
        ct = np.ascontiguousarray(cnt[:, inv])
        xg = np.zeros((r_max, k_pad), np.float32)
        xg[:r, :x_nodes.shape[1]] = x_nodes[cols]
        xgs.append(xg)
        cts.append(ct)
    if any(x is None for x in xgs):
        return None, None, rmax_seen
    return xgs, cts, rmax_seen


def _round_up(x, q):
    return ((x + q - 1) // q) * q


def _prepare(inputs):
    """Host preprocessing: returns (compiled program, per-core in_maps)."""
    inp = {k: np.asarray(v) for k, v in inputs.items()}

    rd_max, rp_max = RD_MAX, RP_MAX
    while True:
        xd_l, cdt_l, seen_d = _prep_branch(
            inp["d_index"], inp["d_ecfps"], inp["d_edge_index"],
            inp["d_edge_weight"], ND, rd_max, DX)
        if xd_l is None:
            rd_max = _round_up(seen_d, 128)
            continue
        break
    while True:
        xp_l, cpt_l, seen_p = _prep_branch(
            inp["p_index"], inp["p_gos"], inp["p_edge_index"],
            inp["p_edge_weight"], NP, rp_max, PX_PAD)
        if xp_l is None:
            rp_max = _round_up(seen_p, 128)
            continue
        break

    f32 = np.float32
    w_pg = np.zeros((PX_PAD, H2), f32)
    w_pg[:PX] = inp["W_pg"]
    w_e1p = np.zeros((FDIM_PAD, H1), f32)
    w_e1p[0:300] = inp["W_e1"][0:300]
    w_e1p[384:1408] = inp["W_e1"][300:1324]
    w_e1p[1408:2432] = inp["W_e1"][1324:2348]
    w_e1p[2432:3456] = inp["W_e1"][2348:3372]

    common = {
        "w_dg": np.ascontiguousarray(inp["W_dg"], dtype=f32),
        "w_pg": w_pg,
        "w_e1": w_e1p,
        "w_e2": np.ascontiguousarray(inp["W_e2"], dtype=f32),
        "w_o1": np.ascontiguousarray(inp["W_o1"], dtype=f32),
        "w_o2": np.ascontiguousarray(inp["W_o2"], dtype=f32),
        "b_dg": np.ascontiguousarray(inp["b_dg"], dtype=f32),
        "b_pg": np.ascontiguousarray(inp["b_pg"], dtype=f32),
        "g_e1": np.ascontiguousarray(inp["g_e1"], dtype=f32),
        "be_e1": np.ascontiguousarray(inp["be_e1"], dtype=f32),
        "g_e2": np.ascontiguousarray(inp["g_e2"], dtype=f32),
        "be_e2": np.ascontiguousarray(inp["be_e2"], dtype=f32),
        "b_o1": np.ascontiguousarray(inp["b_o1"], dtype=f32),
        "g_o": np.ascontiguousarray(inp["g_o"], dtype=f32),
        "be_o": np.ascontiguousarray(inp["be_o"], dtype=f32),
        "b_o2": np.ascontiguousarray(inp["b_o2"], dtype=f32),
    }

    d_vecs = np.asarray(inp["d_vecs"], dtype=f32)
    p_emb = np.asarray(inp["p_embeddings"], dtype=f32)

    in_maps = []
    for c in range(N_CORES):
        rows = slice(c * BL, (c + 1) * BL)
        fin = np.zeros((FIN_ROWS, BL), f32)
        fin[0:300] = d_vecs[rows].T
        fin[384:1408] = p_emb[rows].T
        m = dict(common)
        m.update({
            "xd": xd_l[c], "cdt": cdt_l[c],
            "xp": xp_l[c], "cpt": cpt_l[c],
            "fin": fin,
        })
        in_maps.append(m)

    nc = _get_program(rd_max, rp_max)
    return nc, in_maps


def _assemble(results):
    f32 = np.float32
    y = np.empty((B, 1), f32)
    feature = np.empty((B, H2), f32)
    for c in range(N_CORES):
        y[c * BL:(c + 1) * BL, 0] = results[c]["yt_out"][0]
        feature[c * BL:(c + 1) * BL] = results[c]["ft_out"].T
    return y, feature


def kernel(**inputs):
    nc, in_maps = _prepare(inputs)
    res = run_bass_kernel_spmd(nc, in_maps, core_ids=list(range(N_CORES)))
    return _assemble(res.results)


# revision 7
# speedup vs baseline: 1.0717x; 1.0717x over previous
"""Trainium2 Bass kernel for nn_FC_72232759984564 (GNN message passing + MLP).

Strategy (8 NeuronCores, SPMD, batch-parallel):
  * The final [d_index]/[p_index] gathers make all but ~4k of the 50k/20k
    GCN output rows dead.  For each core's 512-row batch shard the host
    builds, per GCN branch, the set of needed node rows (gather targets +
    their in-edge sources), a gathered node-feature matrix Xg [R, F], and
    a dense coefficient matrix C^T [R, 512] encoding the symmetric-norm
    weighted adjacency (+ self loops), so that the branch output is
    leaky(W^T @ (Xg^T @ C^T) + b)  — two dense matmuls per branch.
  * All activations live transposed on-chip: [features(partitions), batch].
    BatchNorm (training mode, batch axis) needs global batch stats: each
    core computes per-feature sum/sumsq and a small AllReduce (~KB) merges
    them; a dummy AllReduce issued at kernel start absorbs the first-
    collective setup cost while the PE array is busy with the GCN branches.
    BN1 stats ship as two half-AllReduces so the latency hides under the
    second half of the e1 matmuls.
  * Matmuls run in float32r (full fp32 data, ~2e-4 matmul rounding, full
    PE rate).  Feature concat boundaries are padded to multiples of 128 on
    the host (W_e1 rows are permuted to match), so every tile is dense.
  * DMA issue is spread across engine queues (sync: streamed weight
    blocks, gpsimd: residents, scalar: collective bounces/outputs) so the
    ~1.2us/issue descriptor generation doesn't serialize the pipeline.
"""

import numpy as np

import concourse.bacc as bacc
import concourse.mybir as mybir
import concourse.tile as tile
from concourse.bass_utils import run_bass_kernel_spmd

N_CORES = 8
B = 4096
BL = B // N_CORES            # 512 batch rows per core

ND, NP = 50000, 20000        # node counts
DX = 1024                    # d_ecfps feature dim (already 128-aligned)
PX, PX_PAD = 2812, 2816      # p_gos feature dim, padded
FIN_ROWS = 1408              # 384 (dv padded to 3 tiles) + 1024 (pe)
H1, H2, H3 = 2048, 1024, 256
FDIM_PAD = 3456              # 384 + 1024 + 1024 + 1024 = 27 * 128
RD_MAX, RP_MAX = 1536, 2560  # padded gathered-row counts (>=8 sigma margin)
EPS = 1e-5

F32 = mybir.dt.float32
F32R = mybir.dt.float32r
AF = mybir.ActivationFunctionType
AX = mybir.AxisListType
RG = [list(range(N_CORES))]

MD = DX // 128               # 8   (G_d row tiles)
MP = PX_PAD // 128           # 22  (G_p row tiles)
MH = H2 // 128               # 8   (branch output tiles)
KF = FDIM_PAD // 128         # 27
M1 = H1 // 128               # 16
M2 = H2 // 128               # 8
M3 = H3 // 128               # 2

_PROGRAMS = {}


def _col_block(ap, m, kt):
    """DRAM AP for a [kt*128, 128] column block m, tiled [128, kt, 128]."""
    return ap[:, m * 128:(m + 1) * 128].rearrange("(k p) c -> p k c", p=128)


def _build_program(rd_max, rp_max):
    kd, kp = rd_max // 128, rp_max // 128
    nc = bacc.Bacc("TRN2", target_bir_lowering=False, debug=False,
                   num_devices=N_CORES)

    def din(name, shape, dt=F32R):
        return nc.dram_tensor(name, shape, dt, kind="ExternalInput").ap()

    xd = din("xd", [rd_max, DX])
    cdt = din("cdt", [rd_max, BL])
    xp = din("xp", [rp_max, PX_PAD])
    cpt = din("cpt", [rp_max, BL])
    fin = din("fin", [FIN_ROWS, BL])
    w_dg = din("w_dg", [DX, H2])
    w_pg = din("w_pg", [PX_PAD, H2])
    w_e1 = din("w_e1", [FDIM_PAD, H1])
    w_e2 = din("w_e2", [H1, H2])
    w_o1 = din("w_o1", [H2, H3])
    w_o2 = din("w_o2", [H3, 1])
    b_dg = din("b_dg", [H2], F32)
    b_pg = din("b_pg", [H2], F32)
    g_e1 = din("g_e1", [H1], F32)
    be_e1 = din("be_e1", [H1], F32)
    g_e2 = din("g_e2", [H2], F32)
    be_e2 = din("be_e2", [H2], F32)
    b_o1 = din("b_o1", [H3], F32)
    g_o = din("g_o", [H3], F32)
    be_o = din("be_o", [H3], F32)
    b_o2 = din("b_o2", [1], F32)

    ft_out = nc.dram_tensor("ft_out", [H2, BL], F32R, kind="ExternalOutput").ap()
    yt_out = nc.dram_tensor("yt_out", [1, BL], F32, kind="ExternalOutput").ap()
    dbg_out = nc.dram_tensor("dbg_out", [128, 1], F32, kind="ExternalOutput").ap()

    with tile.TileContext(nc) as tc:
        with (
            tc.tile_pool(name="const", bufs=1) as cp,
            tc.tile_pool(name="ws", bufs=2) as ws,
            tc.tile_pool(name="big", bufs=1) as bg,
            tc.tile_pool(name="psum", bufs=4, space="PSUM") as pp,
            tc.tile_pool(name="dram", bufs=1, space="DRAM") as dp,
        ):
            # ---- dummy collective: absorb first-collective setup early ----
            zt = cp.tile([128, 1], F32, tag="zt")
            nc.vector.memset(zt[:], 0.0)
            ar0_in = dp.tile([128, 1], F32, tag="ar0i")
            ar0_out = dp.tile([128, 1], F32, tag="ar0o")
            nc.scalar.dma_start(ar0_in[:], zt[:])
            nc.gpsimd.collective_compute(
                "AllReduce", mybir.AluOpType.add, replica_groups=RG,
                ins=[ar0_in.opt()], outs=[ar0_out.opt()])
            nc.scalar.dma_start(dbg_out, ar0_out[:])

            # ---- branch D inputs first: they gate the first matmuls ----
            ct_t = bg.tile([128, kd, BL], F32R, tag="ct")
            nc.sync.dma_start(ct_t[:], cdt.rearrange("(k p) b -> p k b", p=128))

            # residents needed later: issue early on the gpsimd queue
            ft_t = bg.tile([128, KF, BL], F32R, tag="ft")
            nc.gpsimd.dma_start(ft_t[:, 0:11, :],
                                fin.rearrange("(t p) b -> p t b", p=128))
            w_o1_t = cp.tile([128, M2, H3], F32R, tag="w_o1")
            nc.gpsimd.dma_start(w_o1_t[:],
                                w_o1.rearrange("(k p) c -> p k c", p=128))
            w_o2_t = cp.tile([128, M3, 1], F32R, tag="w_o2")
            nc.gpsimd.dma_start(w_o2_t[:],
                                w_o2.rearrange("(k p) o -> p k o", p=128))

            def vec_tile(src, nm, name):
                t = cp.tile([128, nm], F32, tag=name)
                nc.gpsimd.dma_start(t[:], src.rearrange("(m p) -> p m", p=128))
                return t

            b_dg_t = vec_tile(b_dg, MH, "b_dg")
            b_pg_t = vec_tile(b_pg, MH, "b_pg")
            g_e1_t = vec_tile(g_e1, M1, "g_e1")
            be_e1_t = vec_tile(be_e1, M1, "be_e1")
            g_e2_t = vec_tile(g_e2, M2, "g_e2")
            be_e2_t = vec_tile(be_e2, M2, "be_e2")
            b_o1_t = vec_tile(b_o1, M3, "b_o1")
            g_o_t = vec_tile(g_o, M3, "g_o")
            be_o_t = vec_tile(be_o, M3, "be_o")
            b_o2_t = cp.tile([1, 1], F32, tag="b_o2")
            nc.gpsimd.dma_start(b_o2_t[:], b_o2.unsqueeze(1))

            sq = cp.tile([128, BL], F32, tag="sq")

            # ================= branch D =================
            g_t = bg.tile([128, MD, BL], F32R, tag="g")
            for m in range(MD):
                xdb = ws.tile([128, kd, 128], F32R, tag="wblk")
                nc.sync.dma_start(xdb[:], _col_block(xd, m, kd))
                ps = pp.tile([128, BL], F32, tag="mm")
                for k in range(kd):
                    nc.tensor.matmul(ps[:], xdb[:, k, :], ct_t[:, k, :],
                                     start=(k == 0), stop=(k == kd - 1))
                nc.vector.tensor_copy(g_t[:, m, :], ps[:])

            # prefetch branch-P coefficient matrix into the same slot as cdt
            cpt_t = bg.tile([128, kp, BL], F32R, tag="ct")
            nc.gpsimd.dma_start(cpt_t[:], cpt.rearrange("(k p) b -> p k b", p=128))

            for m in range(MH):
                wdb = ws.tile([128, MD, 128], F32R, tag="wblk")
                nc.sync.dma_start(wdb[:], _col_block(w_dg, m, MD))
                ps = pp.tile([128, BL], F32, tag="mm")
                for k in range(MD):
                    nc.tensor.matmul(ps[:], wdb[:, k, :], g_t[:, k, :],
                                     start=(k == 0), stop=(k == MD - 1))
                nc.scalar.activation(ft_t[:, 11 + m, :], ps[:], AF.Lrelu,
                                     bias=b_dg_t[:, m:m + 1], alpha=0.01)

            # ================= branch P =================
            gp_t = bg.tile([128, MP, BL], F32R, tag="g")
            for m in range(MP):
                xpb = ws.tile([128, kp, 128], F32R, tag="wblk")
                nc.sync.dma_start(xpb[:], _col_block(xp, m, kp))
                ps = pp.tile([128, BL], F32, tag="mm")
                for k in range(kp):
                    nc.tensor.matmul(ps[:], xpb[:, k, :], cpt_t[:, k, :],
                                     start=(k == 0), stop=(k == kp - 1))
                nc.vector.tensor_copy(gp_t[:, m, :], ps[:])
            for m in range(MH):
                wpb = ws.tile([128, MP, 128], F32R, tag="wblk")
                nc.sync.dma_start(wpb[:], _col_block(w_pg, m, MP))
                ps = pp.tile([128, BL], F32, tag="mm")
                for k in range(MP):
                    nc.tensor.matmul(ps[:], wpb[:, k, :], gp_t[:, k, :],
                                     start=(k == 0), stop=(k == MP - 1))
                nc.scalar.activation(ft_t[:, 19 + m, :], ps[:], AF.Lrelu,
                                     bias=b_pg_t[:, m:m + 1], alpha=0.01)

            # ================= MLP =================
            def bn_scale_shift(stats_g, nm, g_t_, be_t_, goff, name):
                """stats_g [128, 2*nm] (sums 0:nm, sumsqs nm:2nm) ->
                scale/shift [128, nm] using gamma/beta columns goff:goff+nm."""
                mean = cp.tile([128, nm], F32, tag=f"{name}_mean")
                var = cp.tile([128, nm], F32, tag=f"{name}_var")
                scl = cp.tile([128, nm], F32, tag=f"{name}_scl")
                shf = cp.tile([128, nm], F32, tag=f"{name}_shf")
                nc.vector.tensor_scalar_mul(mean[:], stats_g[:, 0:nm], 1.0 / B)
                nc.vector.tensor_mul(var[:], mean[:], mean[:])
                nc.vector.tensor_scalar_mul(scl[:], stats_g[:, nm:2 * nm], 1.0 / B)
                nc.vector.tensor_sub(var[:], scl[:], var[:])
                nc.vector.tensor_scalar_add(var[:], var[:], EPS)
                nc.scalar.activation(var[:], var[:], AF.Sqrt)
                nc.vector.reciprocal(var[:], var[:])
                nc.vector.tensor_mul(scl[:], var[:], g_t_[:, goff:goff + nm])
                nc.vector.tensor_mul(shf[:], mean[:], scl[:])
                nc.vector.tensor_sub(shf[:], be_t_[:, goff:goff + nm], shf[:])
                return scl, shf

            def all_reduce_stats(stats, nm, name):
                ar_in = dp.tile([128, 2 * nm], F32, tag=f"{name}_i")
                ar_out = dp.tile([128, 2 * nm], F32, tag=f"{name}_o")
                nc.scalar.dma_start(ar_in[:], stats[:])
                nc.gpsimd.collective_compute(
                    "AllReduce", mybir.AluOpType.add, replica_groups=RG,
                    ins=[ar_in.opt()], outs=[ar_out.opt()])
                stats_g = cp.tile([128, 2 * nm], F32, tag=f"{name}_g")
                nc.scalar.dma_start(stats_g[:], ar_out[:])
                return stats_g

            # ---- layer e1 (stats in two halves to hide AR latency) ----
            HM = M1 // 2
            stats1a = cp.tile([128, 2 * HM], F32, tag="stats1a")
            stats1b = cp.tile([128, 2 * HM], F32, tag="stats1b")
            z1_t = bg.tile([128, M1, BL], F32R, tag="ct")
            stats_g1 = [None, None]
            for m in range(M1):
                wb = ws.tile([128, KF, 128], F32R, tag="wblk")
                nc.sync.dma_start(wb[:], _col_block(w_e1, m, KF))
                ps = pp.tile([128, BL], F32, tag="mm")
                for k in range(KF):
                    nc.tensor.matmul(ps[:], wb[:, k, :], ft_t[:, k, :],
                                     start=(k == 0), stop=(k == KF - 1))
                nc.vector.tensor_copy(z1_t[:, m, :], ps[:])
                st, mm = (stats1a, m) if m < HM else (stats1b, m - HM)
                nc.vector.reduce_sum(st[:, mm:mm + 1], ps[:], axis=AX.X)
                nc.scalar.activation(sq[:], ps[:], AF.Square,
                                     accum_out=st[:, HM + mm:HM + mm + 1])
                if m == HM - 1:
                    stats_g1[0] = all_reduce_stats(stats1a, HM, "ar1a")
            stats_g1[1] = all_reduce_stats(stats1b, HM, "ar1b")
            for h in range(2):
                scl1, shf1 = bn_scale_shift(stats_g1[h], HM, g_e1_t, be_e1_t,
                                            h * HM, f"bn1{h}")
                for mm in range(HM):
                    m = h * HM + mm
                    nc.scalar.activation(z1_t[:, m, :], z1_t[:, m, :], AF.Lrelu,
                                         bias=shf1[:, mm:mm + 1],
                                         scale=scl1[:, mm:mm + 1], alpha=0.01)

            # ---- layer e2 ----
            stats2 = cp.tile([128, 2 * M2], F32, tag="stats2")
            z2_t = bg.tile([128, M2, BL], F32R, tag="g")
            f_t = bg.tile([128, M2, BL], F32R, tag="ct")
            for m in range(M2):
                wb = ws.tile([128, M1, 128], F32R, tag="wblk")
                nc.sync.dma_start(wb[:], _col_block(w_e2, m, M1))
                ps = pp.tile([128, BL], F32, tag="mm")
                for k in range(M1):
                    nc.tensor.matmul(ps[:], wb[:, k, :], z1_t[:, k, :],
                                     start=(k == 0), stop=(k == M1 - 1))
                nc.vector.tensor_copy(z2_t[:, m, :], ps[:])
                nc.vector.reduce_sum(stats2[:, m:m + 1], ps[:], axis=AX.X)
                nc.scalar.activation(sq[:], ps[:], AF.Square,
                                     accum_out=stats2[:, M2 + m:M2 + m + 1])
            stats2_g = all_reduce_stats(stats2, M2, "ar2")
            scl2, shf2 = bn_scale_shift(stats2_g, M2, g_e2_t, be_e2_t, 0, "bn2")
            for m in range(M2):
                nc.scalar.activation(f_t[:, m, :], z2_t[:, m, :], AF.Lrelu,
                                     bias=shf2[:, m:m + 1],
                                     scale=scl2[:, m:m + 1], alpha=0.01)
            nc.scalar.dma_start(ft_out.rearrange("(m p) b -> p m b", p=128),
                                f_t[:])

            # ---- output head ----
            stats3 = cp.tile([128, 2 * M3], F32, tag="stats3")
            ot_t = cp.tile([128, M3, BL], F32, tag="ot")
            for m in range(M3):
                ps = pp.tile([128, BL], F32, tag="mm")
                for k in range(M2):
                    nc.tensor.matmul(ps[:], w_o1_t[:, k, m * 128:(m + 1) * 128],
                                     f_t[:, k, :],
                                     start=(k == 0), stop=(k == M2 - 1))
                nc.scalar.activation(ot_t[:, m, :], ps[:], AF.Lrelu,
                                     bias=b_o1_t[:, m:m + 1], alpha=0.01)
                nc.vector.reduce_sum(stats3[:, m:m + 1], ot_t[:, m, :], axis=AX.X)
                nc.scalar.activation(sq[:], ot_t[:, m, :], AF.Square,
                                     accum_out=stats3[:, M3 + m:M3 + m + 1])
            stats3_g = all_reduce_stats(stats3, M3, "ar3")
            scl3, shf3 = bn_scale_shift(stats3_g, M3, g_o_t, be_o_t, 0, "bn3")
            on_t = cp.tile([128, M3, BL], F32R, tag="on")
            for m in range(M3):
                nc.scalar.activation(on_t[:, m, :], ot_t[:, m, :], AF.Identity,
                                     bias=shf3[:, m:m + 1],
                                     scale=scl3[:, m:m + 1])
            ps_y = pp.tile([128, BL], F32, tag="mm")
            for k in range(M3):
                nc.tensor.matmul(ps_y[0:1, :], w_o2_t[:, k, :], on_t[:, k, :],
                                 start=(k == 0), stop=(k == M3 - 1))
            yt_t = cp.tile([1, BL], F32, tag="yt")
            nc.scalar.activation(yt_t[:], ps_y[0:1, :], AF.Identity,
                                 bias=b_o2_t[0:1, 0:1])
            nc.scalar.dma_start(yt_out, yt_t[:])

    nc.compile()
    return nc


def _get_program(rd_max, rp_max):
    key = (rd_max, rp_max)
    if key not in _PROGRAMS:
        _PROGRAMS[key] = _build_program(rd_max, rp_max)
    return _PROGRAMS[key]


def _prep_branch(index_full, x_nodes, edge_index, edge_w, n_nodes, r_max, k_pad):
    """Per-core gathered node features + transposed coefficient matrices."""
    src = np.asarray(edge_index[0], dtype=np.int64)
    dst = np.asarray(edge_index[1], dtype=np.int64)
    w = np.asarray(edge_w, dtype=np.float32)
    index_full = np.asarray(index_full, dtype=np.int64)
    x_nodes = np.asarray(x_nodes, dtype=np.float32)

    deg = (np.bincount(dst, weights=w.astype(np.float64), minlength=n_nodes)
           .astype(np.float32) + np.float32(1.0))
    dis = np.float32(1.0) / np.sqrt(deg)
    norm = dis[src] * w * dis[dst]

    xgs, cts, rmax_seen = [], [], 0
    for c in range(N_CORES):
        idx = index_full[c * BL:(c + 1) * BL]
        uniq, inv = np.unique(idx, return_inverse=True)
        sel = np.isin(dst, uniq)
        es, ed, en = src[sel], dst[sel], norm[sel]
        cols = np.union1d(es, uniq)
        r = len(cols)
        rmax_seen = max(rmax_seen, r)
        if r > r_max:
            return None, None, rmax_seen
        cnt = np.zeros((r_max, len(uniq)), np.float32)
        np.add.at(cnt, (np.searchsorted(cols, es), np.searchsorted(uniq, ed)), en)
        cnt[np.searchsorted(cols, uniq), np.arange(len(uniq))] += dis[uniq] ** 2
        ct = np.ascontiguousarray(cnt[:, inv])
        xg = np.zeros((r_max, k_pad), np.float32)
        xg[:r, :x_nodes.shape[1]] = x_nodes[cols]
        xgs.append(xg)
        cts.append(ct)
    return xgs, cts, rmax_seen


def _round_up(x, q):
    return ((x + q - 1) // q) * q


def _prepare(inputs):
    """Host preprocessing: returns (compiled program, per-core in_maps)."""
    inp = {k: np.asarray(v) for k, v in inputs.items()}

    rd_max, rp_max = RD_MAX, RP_MAX
    while True:
        xd_l, cdt_l, seen_d = _prep_branch(
            inp["d_index"], inp["d_ecfps"], inp["d_edge_index"],
            inp["d_edge_weight"], ND, rd_max, DX)
        if xd_l is None:
            rd_max = _round_up(seen_d, 128)
            continue
        break
    while True:
        xp_l, cpt_l, seen_p = _prep_branch(
            inp["p_index"], inp["p_gos"], inp["p_edge_index"],
            inp["p_edge_weight"], NP, rp_max, PX_PAD)
        if xp_l is None:
            rp_max = _round_up(seen_p, 128)
            continue
        break

    f32 = np.float32
    w_pg = np.zeros((PX_PAD, H2), f32)
    w_pg[:PX] = inp["W_pg"]
    w_e1p = np.zeros((FDIM_PAD, H1), f32)
    w_e1p[0:300] = inp["W_e1"][0:300]
    w_e1p[384:1408] = inp["W_e1"][300:1324]
    w_e1p[1408:2432] = inp["W_e1"][1324:2348]
    w_e1p[2432:3456] = inp["W_e1"][2348:3372]

    common = {
        "w_dg": np.ascontiguousarray(inp["W_dg"], dtype=f32),
        "w_pg": w_pg,
        "w_e1": w_e1p,
        "w_e2": np.ascontiguousarray(inp["W_e2"], dtype=f32),
        "w_o1": np.ascontiguousarray(inp["W_o1"], dtype=f32),
        "w_o2": np.ascontiguousarray(inp["W_o2"], dtype=f32),
        "b_dg": np.ascontiguousarray(inp["b_dg"], dtype=f32),
        "b_pg": np.ascontiguousarray(inp["b_pg"], dtype=f32),
        "g_e1": np.ascontiguousarray(inp["g_e1"], dtype=f32),
        "be_e1": np.ascontiguousarray(inp["be_e1"], dtype=f32),
        "g_e2": np.ascontiguousarray(inp["g_e2"], dtype=f32),
        "be_e2": np.ascontiguousarray(inp["be_e2"], dtype=f32),
        "b_o1": np.ascontiguousarray(inp["b_o1"], dtype=f32),
        "g_o": np.ascontiguousarray(inp["g_o"], dtype=f32),
        "be_o": np.ascontiguousarray(inp["be_o"], dtype=f32),
        "b_o2": np.ascontiguousarray(inp["b_o2"], dtype=f32),
    }

    d_vecs = np.asarray(inp["d_vecs"], dtype=f32)
    p_emb = np.asarray(inp["p_embeddings"], dtype=f32)

    in_maps = []
    for c in range(N_CORES):
        rows = slice(c * BL, (c + 1) * BL)
        fin = np.zeros((FIN_ROWS, BL), f32)
        fin[0:300] = d_vecs[rows].T
        fin[384:1408] = p_emb[rows].T
        m = dict(common)
        m.update({
            "xd": xd_l[c], "cdt": cdt_l[c],
            "xp": xp_l[c], "cpt": cpt_l[c],
            "fin": fin,
        })
        in_maps.append(m)

    nc = _get_program(rd_max, rp_max)
    return nc, in_maps


def _assemble(results):
    f32 = np.float32
    y = np.empty((B, 1), f32)
    feature = np.empty((B, H2), f32)
    for c in range(N_CORES):
        y[c * BL:(c + 1) * BL, 0] = results[c]["yt_out"][0]
        feature[c * BL:(c + 1) * BL] = results[c]["ft_out"].T
    return y, feature


def kernel(**inputs):
    nc, in_maps = _prepare(inputs)
    res = run_bass_kernel_spmd(nc, in_maps, core_ids=list(range(N_CORES)))
    return _assemble(res.results)


# revision 13
# speedup vs baseline: 1.2340x; 1.1514x over previous
"""Trainium2 Bass kernel for nn_FC_72232759984564 (GNN message passing + MLP).

Strategy (8 NeuronCores, SPMD, batch-parallel):
  * The final [d_index]/[p_index] gathers make all but ~4k of the 50k/20k
    GCN output rows dead.  For each core's 512-row batch shard the host
    builds, per GCN branch, the set of needed node rows (gather targets +
    their in-edge sources), a gathered node-feature matrix Xg [R, F], and
    a dense coefficient matrix C^T [R, 512] encoding the symmetric-norm
    weighted adjacency (+ self loops), so that the branch output is
    leaky(W^T @ (Xg^T @ C^T) + b)  — two dense matmuls per branch.
  * All activations live transposed on-chip: [features(partitions), batch].
    BatchNorm (training mode, batch axis) needs global batch stats: each
    core computes per-feature sum/sumsq and a small AllReduce (~KB) merges
    them; a dummy AllReduce issued at kernel start absorbs the first-
    collective setup cost while the PE array is busy with the GCN branches.
    BN1 stats ship as two uneven AllReduces (12+4 chunks): the first hides
    under the remaining e1 matmuls, the second under a k-split first pass
    of the e2 matmuls.
  * Matmuls run in float32r (full fp32 data, ~2e-4 matmul rounding, full
    PE rate).  Feature concat boundaries are padded to multiples of 128 on
    the host (W_e1 rows are permuted to match), so every tile is dense.
  * Every DRAM input is pre-blocked on the host into its exact SBUF tile
    layout ([m][partition][k][col]) so each DMA is one fully-contiguous
    read at HBM line rate; DMA issue is spread across engine queues.
"""

import numpy as np

import concourse.bacc as bacc
import concourse.mybir as mybir
import concourse.tile as tile
from concourse.bass_utils import run_bass_kernel_spmd

N_CORES = 8
B = 4096
BL = B // N_CORES            # 512 batch rows per core

ND, NP = 50000, 20000        # node counts
DX = 1024                    # d_ecfps feature dim (already 128-aligned)
PX, PX_PAD = 2812, 2816      # p_gos feature dim, padded
FIN_ROWS = 1408              # 384 (dv padded to 3 tiles) + 1024 (pe)
H1, H2, H3 = 2048, 1024, 256
FDIM_PAD = 3456              # 384 + 1024 + 1024 + 1024 = 27 * 128
RD_MAX, RP_MAX = 1536, 2560  # padded gathered-row counts (>=8 sigma margin)
EPS = 1e-5

F32 = mybir.dt.float32
F32R = mybir.dt.float32r
AF = mybir.ActivationFunctionType
AX = mybir.AxisListType
RG = [list(range(N_CORES))]

MD = DX // 128               # 8   (G_d row tiles)
MP = PX_PAD // 128           # 22  (G_p row tiles)
MH = H2 // 128               # 8   (branch output tiles)
KF = FDIM_PAD // 128         # 27
M1 = H1 // 128               # 16
M2 = H2 // 128               # 8
M3 = H3 // 128               # 2
S1A = 12                     # BN1 first-AllReduce chunk count (of M1)
K2A = 12                     # e2 first-pass contraction tiles (of M1)

_PROGRAMS = {}


def _build_program(rd_max, rp_max):
    kd, kp = rd_max // 128, rp_max // 128
    nc = bacc.Bacc("TRN2", target_bir_lowering=False, debug=False,
                   num_devices=N_CORES)

    def din(name, shape, dt=F32R):
        return nc.dram_tensor(name, shape, dt, kind="ExternalInput").ap()

    # all inputs pre-blocked on host into SBUF tile layout
    xd = din("xd", [MD, 128, kd, 128])
    cdt = din("cdt", [128, kd, BL])
    xp = din("xp", [MP, 128, kp, 128])
    cpt = din("cpt", [128, kp, BL])
    fin = din("fin", [128, 11, BL])
    w_dg = din("w_dg", [MH, 128, MD, 128])
    w_pg = din("w_pg", [MH, 128, MP, 128])
    w_e1 = din("w_e1", [M1, 128, KF, 128])
    w_e2a = din("w_e2a", [M2, 128, K2A, 128])
    w_e2b = din("w_e2b", [128, M2, M1 - K2A, 128])
    w_o1 = din("w_o1", [128, M2, H3])
    w_o2 = din("w_o2", [128, M3, 1])
    b_dg = din("b_dg", [128, MH], F32)
    b_pg = din("b_pg", [128, MH], F32)
    g_e1 = din("g_e1", [128, M1], F32)
    be_e1 = din("be_e1", [128, M1], F32)
    g_e2 = din("g_e2", [128, M2], F32)
    be_e2 = din("be_e2", [128, M2], F32)
    b_o1 = din("b_o1", [128, M3], F32)
    g_o = din("g_o", [128, M3], F32)
    be_o = din("be_o", [128, M3], F32)
    b_o2 = din("b_o2", [1, 1], F32)

    ft_out = nc.dram_tensor("ft_out", [H2, BL], F32R, kind="ExternalOutput").ap()
    yt_out = nc.dram_tensor("yt_out", [1, BL], F32, kind="ExternalOutput").ap()
    dbg_out = nc.dram_tensor("dbg_out", [128, 1], F32, kind="ExternalOutput").ap()

    with tile.TileContext(nc) as tc:
        with (
            tc.tile_pool(name="const", bufs=1) as cp,
            tc.tile_pool(name="ws", bufs=2) as ws,
            tc.tile_pool(name="big", bufs=1) as bg,
            tc.tile_pool(name="psum", bufs=8, space="PSUM") as pp,
            tc.tile_pool(name="dram", bufs=1, space="DRAM") as dp,
        ):
            # ---- dummy collective: absorb first-collective setup early ----
            zt = cp.tile([128, 1], F32, tag="zt")
            nc.vector.memset(zt[:], 0.0)
            ar0_in = dp.tile([128, 1], F32, tag="ar0i")
            ar0_out = dp.tile([128, 1], F32, tag="ar0o")
            nc.scalar.dma_start(ar0_in[:], zt[:])
            nc.gpsimd.collective_compute(
                "AllReduce", mybir.AluOpType.add, replica_groups=RG,
                ins=[ar0_in.opt()], outs=[ar0_out.opt()])
            nc.scalar.dma_start(dbg_out, ar0_out[:])

            # ---- branch D inputs first: they gate the first matmuls ----
            ct_t = bg.tile([128, kd, BL], F32R, tag="ct")
            nc.sync.dma_start(ct_t[:], cdt)

            # residents needed later: issue early on the gpsimd queue
            ft_t = bg.tile([128, KF, BL], F32R, tag="ft")
            nc.gpsimd.dma_start(ft_t[:, 0:11, :], fin)
            w_o1_t = cp.tile([128, M2, H3], F32R, tag="w_o1")
            nc.gpsimd.dma_start(w_o1_t[:], w_o1)
            w_o2_t = cp.tile([128, M3, 1], F32R, tag="w_o2")
            nc.gpsimd.dma_start(w_o2_t[:], w_o2)

            def vec_tile(src, nm, name):
                t = cp.tile([128, nm], F32, tag=name)
                nc.gpsimd.dma_start(t[:], src)
                return t

            b_dg_t = vec_tile(b_dg, MH, "b_dg")
            b_pg_t = vec_tile(b_pg, MH, "b_pg")
            g_e1_t = vec_tile(g_e1, M1, "g_e1")
            be_e1_t = vec_tile(be_e1, M1, "be_e1")
            g_e2_t = vec_tile(g_e2, M2, "g_e2")
            be_e2_t = vec_tile(be_e2, M2, "be_e2")
            b_o1_t = vec_tile(b_o1, M3, "b_o1")
            g_o_t = vec_tile(g_o, M3, "g_o")
            be_o_t = vec_tile(be_o, M3, "be_o")
            b_o2_t = cp.tile([1, 1], F32, tag="b_o2")
            nc.gpsimd.dma_start(b_o2_t[:], b_o2)

            sq = cp.tile([128, BL], F32, tag="sq")

            # ================= branch D =================
            g_t = bg.tile([128, MD, BL], F32R, tag="g")
            for m in range(MD):
                xdb = ws.tile([128, kd, 128], F32R, tag="wblk")
                nc.sync.dma_start(xdb[:], xd[m])
                ps = pp.tile([128, BL], F32, tag="mm")
                for k in range(kd):
                    nc.tensor.matmul(ps[:], xdb[:, k, :], ct_t[:, k, :],
                                     start=(k == 0), stop=(k == kd - 1))
                nc.vector.tensor_copy(g_t[:, m, :], ps[:])

            # prefetch branch-P coefficient matrix into the same slot as cdt
            cpt_t = bg.tile([128, kp, BL], F32R, tag="ct")
            nc.gpsimd.dma_start(cpt_t[:], cpt)

            for m in range(MH):
                wdb = ws.tile([128, MD, 128], F32R, tag="wblk")
                nc.sync.dma_start(wdb[:], w_dg[m])
                ps = pp.tile([128, BL], F32, tag="mm")
                for k in range(MD):
                    nc.tensor.matmul(ps[:], wdb[:, k, :], g_t[:, k, :],
                                     start=(k == 0), stop=(k == MD - 1))
                nc.scalar.activation(ft_t[:, 11 + m, :], ps[:], AF.Lrelu,
                                     bias=b_dg_t[:, m:m + 1], alpha=0.01)

            # ================= branch P =================
            gp_t = bg.tile([128, MP, BL], F32R, tag="g")
            for m in range(MP):
                xpb = ws.tile([128, kp, 128], F32R, tag="wblk")
                nc.sync.dma_start(xpb[:], xp[m])
                ps = pp.tile([128, BL], F32, tag="mm")
                for k in range(kp):
                    nc.tensor.matmul(ps[:], xpb[:, k, :], cpt_t[:, k, :],
                                     start=(k == 0), stop=(k == kp - 1))
                nc.vector.tensor_copy(gp_t[:, m, :], ps[:])
            for m in range(MH):
                wpb = ws.tile([128, MP, 128], F32R, tag="wblk")
                nc.sync.dma_start(wpb[:], w_pg[m])
                ps = pp.tile([128, BL], F32, tag="mm")
                for k in range(MP):
                    nc.tensor.matmul(ps[:], wpb[:, k, :], gp_t[:, k, :],
                                     start=(k == 0), stop=(k == MP - 1))
                nc.scalar.activation(ft_t[:, 19 + m, :], ps[:], AF.Lrelu,
                                     bias=b_pg_t[:, m:m + 1], alpha=0.01)

            # ================= MLP =================
            def bn_scale_shift(stats_g, nm, g_t_, be_t_, goff, name):
                """stats_g [128, 2*nm] (sums 0:nm, sumsqs nm:2nm) ->
                scale/shift [128, nm] using gamma/beta columns goff:goff+nm."""
                mean = cp.tile([128, nm], F32, tag=f"{name}_mean")
                var = cp.tile([128, nm], F32, tag=f"{name}_var")
                scl = cp.tile([128, nm], F32, tag=f"{name}_scl")
                shf = cp.tile([128, nm], F32, tag=f"{name}_shf")
                nc.vector.tensor_scalar_mul(mean[:], stats_g[:, 0:nm], 1.0 / B)
                nc.vector.tensor_mul(var[:], mean[:], mean[:])
                nc.vector.tensor_scalar_mul(scl[:], stats_g[:, nm:2 * nm], 1.0 / B)
                nc.vector.tensor_sub(var[:], scl[:], var[:])
                nc.vector.tensor_scalar_add(var[:], var[:], EPS)
                nc.scalar.activation(var[:], var[:], AF.Sqrt)
                nc.vector.reciprocal(var[:], var[:])
                nc.vector.tensor_mul(scl[:], var[:], g_t_[:, goff:goff + nm])
                nc.vector.tensor_mul(shf[:], mean[:], scl[:])
                nc.vector.tensor_sub(shf[:], be_t_[:, goff:goff + nm], shf[:])
                return scl, shf

            def all_reduce_stats(stats, nm, name):
                ar_in = dp.tile([128, 2 * nm], F32, tag=f"{name}_i")
                ar_out = dp.tile([128, 2 * nm], F32, tag=f"{name}_o")
                nc.scalar.dma_start(ar_in[:], stats[:])
                nc.gpsimd.collective_compute(
                    "AllReduce", mybir.AluOpType.add, replica_groups=RG,
                    ins=[ar_in.opt()], outs=[ar_out.opt()])
                stats_g = cp.tile([128, 2 * nm], F32, tag=f"{name}_g")
                nc.scalar.dma_start(stats_g[:], ar_out[:])
                return stats_g

            # ---- layer e1 (uneven split stats to hide AR latency) ----
            S1B = M1 - S1A
            stats1a = cp.tile([128, 2 * S1A], F32, tag="stats1a")
            stats1b = cp.tile([128, 2 * S1B], F32, tag="stats1b")
            z1_t = bg.tile([128, M1, BL], F32R, tag="ct")
            stats_g1 = [None, None]
            for m in range(M1):
                wb = ws.tile([128, KF, 128], F32R, tag="wblk")
                nc.sync.dma_start(wb[:], w_e1[m])
                ps = pp.tile([128, BL], F32, tag="mm")
                for k in range(KF):
                    nc.tensor.matmul(ps[:], wb[:, k, :], ft_t[:, k, :],
                                     start=(k == 0), stop=(k == KF - 1))
                nc.vector.tensor_copy(z1_t[:, m, :], ps[:])
                st, mm_, hn = ((stats1a, m, S1A) if m < S1A
                               else (stats1b, m - S1A, S1B))
                nc.vector.reduce_sum(st[:, mm_:mm_ + 1], ps[:], axis=AX.X)
                nc.scalar.activation(sq[:], ps[:], AF.Square,
                                     accum_out=st[:, hn + mm_:hn + mm_ + 1])
                if m == S1A - 1:
                    stats_g1[0] = all_reduce_stats(stats1a, S1A, "ar1a")
            stats_g1[1] = all_reduce_stats(stats1b, S1B, "ar1b")
            for h, (nm, off) in enumerate([(S1A, 0), (S1B, S1A)]):
                scl1, shf1 = bn_scale_shift(stats_g1[h], nm, g_e1_t, be_e1_t,
                                            off, f"bn1{h}")
                for mm_ in range(nm):
                    m = off + mm_
                    nc.scalar.activation(z1_t[:, m, :], z1_t[:, m, :], AF.Lrelu,
                                         bias=shf1[:, mm_:mm_ + 1],
                                         scale=scl1[:, mm_:mm_ + 1], alpha=0.01)

            # ---- layer e2 (two k-passes: first pass only needs h[0:K2A],
            #      which is ready while the second stats AllReduce flies) ----
            stats2 = cp.tile([128, 2 * M2], F32, tag="stats2")
            z2_t = bg.tile([128, M2, BL], F32R, tag="g")
            f_t = bg.tile([128, M2, BL], F32R, tag="ct")
            # pass-B weight slices live in the (now dead) ft slot
            wbb_t = bg.tile([128, M2, M1 - K2A, 128], F32R, tag="ft")
            nc.gpsimd.dma_start(wbb_t[:], w_e2b)
            for m in range(M2):
                wb = ws.tile([128, K2A, 128], F32R, tag="wblk")
                nc.sync.dma_start(wb[:], w_e2a[m])
                ps = pp.tile([128, BL], F32, tag="mm")
                for k in range(K2A):
                    nc.tensor.matmul(ps[:], wb[:, k, :], z1_t[:, k, :],
                                     start=(k == 0), stop=(k == K2A - 1))
                nc.vector.tensor_copy(z2_t[:, m, :], ps[:])
            for m in range(M2):
                ps = pp.tile([128, BL], F32, tag="mm")
                for k in range(K2A, M1):
                    nc.tensor.matmul(ps[:], wbb_t[:, m, k - K2A, :],
                                     z1_t[:, k, :],
                                     start=(k == K2A), stop=(k == M1 - 1))
                nc.vector.tensor_add(z2_t[:, m, :], z2_t[:, m, :], ps[:])
                nc.vector.reduce_sum(stats2[:, m:m + 1], z2_t[:, m, :], axis=AX.X)
                nc.scalar.activation(sq[:], z2_t[:, m, :], AF.Square,
                                     accum_out=stats2[:, M2 + m:M2 + m + 1])
            stats2_g = all_reduce_stats(stats2, M2, "ar2")
            scl2, shf2 = bn_scale_shift(stats2_g, M2, g_e2_t, be_e2_t, 0, "bn2")
            for m in range(M2):
                nc.scalar.activation(f_t[:, m, :], z2_t[:, m, :], AF.Lrelu,
                                     bias=shf2[:, m:m + 1],
                                     scale=scl2[:, m:m + 1], alpha=0.01)
            nc.scalar.dma_start(ft_out.rearrange("(m p) b -> p m b", p=128),
                                f_t[:])

            # ---- output head ----
            stats3 = cp.tile([128, 2 * M3], F32, tag="stats3")
            ot_t = cp.tile([128, M3, BL], F32, tag="ot")
            for m in range(M3):
                ps = pp.tile([128, BL], F32, tag="mm")
                for k in range(M2):
                    nc.tensor.matmul(ps[:], w_o1_t[:, k, m * 128:(m + 1) * 128],
                                     f_t[:, k, :],
                                     start=(k == 0), stop=(k == M2 - 1))
                nc.scalar.activation(ot_t[:, m, :], ps[:], AF.Lrelu,
                                     bias=b_o1_t[:, m:m + 1], alpha=0.01)
                nc.vector.reduce_sum(stats3[:, m:m + 1], ot_t[:, m, :], axis=AX.X)
                nc.scalar.activation(sq[:], ot_t[:, m, :], AF.Square,
                                     accum_out=stats3[:, M3 + m:M3 + m + 1])
            stats3_g = all_reduce_stats(stats3, M3, "ar3")
            scl3, shf3 = bn_scale_shift(stats3_g, M3, g_o_t, be_o_t, 0, "bn3")
            on_t = cp.tile([128, M3, BL], F32R, tag="on")
            for m in range(M3):
                nc.scalar.activation(on_t[:, m, :], ot_t[:, m, :], AF.Identity,
                                     bias=shf3[:, m:m + 1],
                                     scale=scl3[:, m:m + 1])
            ps_y = pp.tile([128, BL], F32, tag="mm")
            for k in range(M3):
                nc.tensor.matmul(ps_y[0:1, :], w_o2_t[:, k, :], on_t[:, k, :],
                                 start=(k == 0), stop=(k == M3 - 1))
            yt_t = cp.tile([1, BL], F32, tag="yt")
            nc.scalar.activation(yt_t[:], ps_y[0:1, :], AF.Identity,
                                 bias=b_o2_t[0:1, 0:1])
            nc.scalar.dma_start(yt_out, yt_t[:])

    nc.compile()
    return nc


def _get_program(rd_max, rp_max):
    key = (rd_max, rp_max)
    if key not in _PROGRAMS:
        _PROGRAMS[key] = _build_program(rd_max, rp_max)
    return _PROGRAMS[key]


def _block(w, kt, mt):
    """[kt*128, mt*128] -> [mt, 128, kt, 128] contiguous (lhsT tile layout)."""
    return np.ascontiguousarray(
        w.reshape(kt, 128, mt, 128).transpose(2, 1, 0, 3))


def _rows_block(x, kt, cols):
    """[kt*128, cols] -> [128, kt, cols] contiguous."""
    return np.ascontiguousarray(x.reshape(kt, 128, cols).transpose(1, 0, 2))


def _vec_block(v, mt):
    """[mt*128] -> [128, mt] contiguous."""
    return np.ascontiguousarray(np.asarray(v, np.float32).reshape(mt, 128).T)


def _prep_branch(index_full, x_nodes, edge_index, edge_w, n_nodes, r_max, k_pad):
    """Per-core gathered node features + transposed coefficient matrices,
    both in blocked device layout."""
    src = np.asarray(edge_index[0], dtype=np.int64)
    dst = np.asarray(edge_index[1], dtype=np.int64)
    w = np.asarray(edge_w, dtype=np.float32)
    index_full = np.asarray(index_full, dtype=np.int64)
    x_nodes = np.asarray(x_nodes, dtype=np.float32)
    kt, mt = r_max // 128, k_pad // 128

    deg = (np.bincount(dst, weights=w.astype(np.float64), minlength=n_nodes)
           .astype(np.float32) + np.float32(1.0))
    dis = np.float32(1.0) / np.sqrt(deg)
    norm = dis[src] * w * dis[dst]

    xgs, cts, rmax_seen = [], [], 0
    for c in range(N_CORES):
        idx = index_full[c * BL:(c + 1) * BL]
        uniq, inv = np.unique(idx, return_inverse=True)
        sel = np.isin(dst, uniq)
        es, ed, en = src[sel], dst[sel], norm[sel]
        cols = np.union1d(es, uniq)
        r = len(cols)
        rmax_seen = max(rmax_seen, r)
        if r > r_max:
            return None, None, rmax_seen
        cnt = np.zeros((r_max, len(uniq)), np.float32)
        np.add.at(cnt, (np.searchsorted(cols, es), np.searchsorted(uniq, ed)), en)
        cnt[np.searchsorted(cols, uniq), np.arange(len(uniq))] += dis[uniq] ** 2
        cts.append(_rows_block(cnt[:, inv], kt, BL))
        xg = np.zeros((r_max, k_pad), np.float32)
        xg[:r, :x_nodes.shape[1]] = x_nodes[cols]
        xgs.append(_block(xg, kt, mt))
    return xgs, cts, rmax_seen


def _round_up(x, q):
    return ((x + q - 1) // q) * q


def _prepare(inputs):
    """Host preprocessing: returns (compiled program, per-core in_maps)."""
    inp = {k: np.asarray(v) for k, v in inputs.items()}

    rd_max, rp_max = RD_MAX, RP_MAX
    while True:
        xd_l, cdt_l, seen_d = _prep_branch(
            inp["d_index"], inp["d_ecfps"], inp["d_edge_index"],
            inp["d_edge_weight"], ND, rd_max, DX)
        if xd_l is None:
            rd_max = _round_up(seen_d, 128)
            continue
        break
    while True:
        xp_l, cpt_l, seen_p = _prep_branch(
            inp["p_index"], inp["p_gos"], inp["p_edge_index"],
            inp["p_edge_weight"], NP, rp_max, PX_PAD)
        if xp_l is None:
            rp_max = _round_up(seen_p, 128)
            continue
        break

    f32 = np.float32
    w_pg = np.zeros((PX_PAD, H2), f32)
    w_pg[:PX] = inp["W_pg"]
    w_e1p = np.zeros((FDIM_PAD, H1), f32)
    w_e1p[0:300] = inp["W_e1"][0:300]
    w_e1p[384:1408] = inp["W_e1"][300:1324]
    w_e1p[1408:2432] = inp["W_e1"][1324:2348]
    w_e1p[2432:3456] = inp["W_e1"][2348:3372]

    common = {
        "w_dg": _block(np.asarray(inp["W_dg"], f32), MD, MH),
        "w_pg": _block(w_pg, MP, MH),
        "w_e1": _block(w_e1p, KF, M1),
        "w_e2a": np.ascontiguousarray(
            _block(np.asarray(inp["W_e2"], f32), M1, M2)[:, :, 0:K2A, :]),
        "w_e2b": np.ascontiguousarray(
            _block(np.asarray(inp["W_e2"], f32), M1, M2)[:, :, K2A:, :]
            .transpose(1, 0, 2, 3)),
        "w_o1": _rows_block(np.asarray(inp["W_o1"], f32), M2, H3),
        "w_o2": _rows_block(np.asarray(inp["W_o2"], f32), M3, 1),
        "b_dg": _vec_block(inp["b_dg"], MH),
        "b_pg": _vec_block(inp["b_pg"], MH),
        "g_e1": _vec_block(inp["g_e1"], M1),
        "be_e1": _vec_block(inp["be_e1"], M1),
        "g_e2": _vec_block(inp["g_e2"], M2),
        "be_e2": _vec_block(inp["be_e2"], M2),
        "b_o1": _vec_block(inp["b_o1"], M3),
        "g_o": _vec_block(inp["g_o"], M3),
        "be_o": _vec_block(inp["be_o"], M3),
        "b_o2": np.asarray(inp["b_o2"], f32).reshape(1, 1),
    }

    d_vecs = np.asarray(inp["d_vecs"], dtype=f32)
    p_emb = np.asarray(inp["p_embeddings"], dtype=f32)

    in_maps = []
    for c in range(N_CORES):
        rows = slice(c * BL, (c + 1) * BL)
        fin = np.zeros((FIN_ROWS, BL), f32)
        fin[0:300] = d_vecs[rows].T
        fin[384:1408] = p_emb[rows].T
        m = dict(common)
        m.update({
            "xd": xd_l[c], "cdt": cdt_l[c],
            "xp": xp_l[c], "cpt": cpt_l[c],
            "fin": _rows_block(fin, 11, BL),
        })
        in_maps.append(m)

    nc = _get_program(rd_max, rp_max)
    return nc, in_maps


def _assemble(results):
    f32 = np.float32
    y = np.empty((B, 1), f32)
    feature = np.empty((B, H2), f32)
    for c in range(N_CORES):
        y[c * BL:(c + 1) * BL, 0] = results[c]["yt_out"][0]
        feature[c * BL:(c + 1) * BL] = results[c]["ft_out"].T
    return y, feature


def kernel(**inputs):
    nc, in_maps = _prepare(inputs)
    res = run_bass_kernel_spmd(nc, in_maps, core_ids=list(range(N_CORES)))
    return _assemble(res.results)


# revision 17
# speedup vs baseline: 1.3604x; 1.1025x over previous
"""Trainium2 Bass kernel for nn_FC_72232759984564 (GNN message passing + MLP).

Strategy (8 NeuronCores, SPMD, batch-parallel):
  * The final [d_index]/[p_index] gathers make all but ~4k of the 50k/20k
    GCN output rows dead.  For each core's 512-row batch shard the host
    builds, per GCN branch, the set of needed node rows (gather targets +
    their in-edge sources), a gathered node-feature matrix Xg [R, F], and
    a dense coefficient matrix C^T [R, 512] encoding the symmetric-norm
    weighted adjacency (+ self loops), so that the branch output is
    leaky(W^T @ (Xg^T @ C^T) + b)  — two dense matmuls per branch.
  * All activations live transposed on-chip: [features(partitions), batch].
    BatchNorm (training mode, batch axis) needs global batch stats: each
    core computes per-feature sum/sumsq and a small AllReduce (~KB) merges
    them; a dummy AllReduce issued at kernel start absorbs the first-
    collective setup cost while the PE array is busy with the GCN branches.
    BN1 stats ship as two uneven AllReduces (12+4 chunks): the first hides
    under the remaining e1 matmuls, the second under a k-split first pass
    of the e2 matmuls.
  * Matmuls run in float32r (full fp32 data, ~2e-4 matmul rounding, full
    PE rate).  Feature concat boundaries are padded to multiples of 128 on
    the host (W_e1 rows are permuted to match), so every tile is dense.
  * Every DRAM input is pre-blocked on the host into its exact SBUF tile
    layout ([m][partition][k][col]) so each DMA is one fully-contiguous
    read at HBM line rate; DMA issue is spread across engine queues.
"""

import numpy as np

import concourse.bacc as bacc
import concourse.mybir as mybir
import concourse.tile as tile
from concourse.bass_utils import run_bass_kernel_spmd

N_CORES = 8
B = 4096
BL = B // N_CORES            # 512 batch rows per core

ND, NP = 50000, 20000        # node counts
DX = 1024                    # d_ecfps feature dim (already 128-aligned)
PX, PX_PAD = 2812, 2816      # p_gos feature dim, padded
FIN_ROWS = 1408              # 384 (dv padded to 3 tiles) + 1024 (pe)
H1, H2, H3 = 2048, 1024, 256
FDIM_PAD = 3456              # 384 + 1024 + 1024 + 1024 = 27 * 128
RD_MAX, RP_MAX = 1536, 2560  # padded gathered-row counts (>=8 sigma margin)
EPS = 1e-5

F32 = mybir.dt.float32
F32R = mybir.dt.float32r
AF = mybir.ActivationFunctionType
AX = mybir.AxisListType
RG = [list(range(N_CORES))]

MD = DX // 128               # 8   (G_d row tiles)
MP = PX_PAD // 128           # 22  (G_p row tiles)
MH = H2 // 128               # 8   (branch output tiles)
KF = FDIM_PAD // 128         # 27
M1 = H1 // 128               # 16
M2 = H2 // 128               # 8
M3 = H3 // 128               # 2
S1A = 12                     # BN1 first-AllReduce chunk count (of M1)
K2A = 12                     # e2 first-pass contraction tiles (of M1)

_PROGRAMS = {}


def _build_program(rd_max, rp_max):
    kd, kp = rd_max // 128, rp_max // 128
    nc = bacc.Bacc("TRN2", target_bir_lowering=False, debug=False,
                   num_devices=N_CORES)

    def din(name, shape, dt=F32R):
        return nc.dram_tensor(name, shape, dt, kind="ExternalInput").ap()

    # all inputs pre-blocked on host into SBUF tile layout
    xd = din("xd", [MD, 128, kd, 128])
    cdt = din("cdt", [128, kd, BL])
    xp = din("xp", [MP, 128, kp, 128])
    cpt = din("cpt", [128, kp, BL])
    fin = din("fin", [128, 11, BL])
    w_dg = din("w_dg", [MH, 128, MD, 128])
    w_pg = din("w_pg", [MH, 128, MP, 128])
    w_e1 = din("w_e1", [M1, 128, KF, 128])
    w_e2a = din("w_e2a", [M2, 128, K2A, 128])
    w_e2b = din("w_e2b", [128, M2, M1 - K2A, 128])
    w_o1 = din("w_o1", [128, M2, H3])
    w_o2 = din("w_o2", [128, M3, 1])
    b_dg = din("b_dg", [128, MH], F32)
    b_pg = din("b_pg", [128, MH], F32)
    g_e1 = din("g_e1", [128, M1], F32)
    be_e1 = din("be_e1", [128, M1], F32)
    g_e2 = din("g_e2", [128, M2], F32)
    be_e2 = din("be_e2", [128, M2], F32)
    b_o1 = din("b_o1", [128, M3], F32)
    g_o = din("g_o", [128, M3], F32)
    be_o = din("be_o", [128, M3], F32)
    b_o2 = din("b_o2", [1, 1], F32)

    ft_out = nc.dram_tensor("ft_out", [H2, BL], F32R, kind="ExternalOutput").ap()
    yt_out = nc.dram_tensor("yt_out", [1, BL], F32, kind="ExternalOutput").ap()
    dbg_out = nc.dram_tensor("dbg_out", [128, 1], F32, kind="ExternalOutput").ap()

    with tile.TileContext(nc) as tc:
        with (
            tc.tile_pool(name="const", bufs=1) as cp,
            tc.tile_pool(name="ws", bufs=3) as ws,
            tc.tile_pool(name="big", bufs=1) as bg,
            tc.tile_pool(name="psum", bufs=8, space="PSUM") as pp,
            tc.tile_pool(name="dram", bufs=1, space="DRAM") as dp,
        ):
            # ---- dummy collective: absorb first-collective setup early ----
            zt = cp.tile([128, 1], F32, tag="zt")
            nc.vector.memset(zt[:], 0.0)
            ar0_in = dp.tile([128, 1], F32, tag="ar0i")
            ar0_out = dp.tile([128, 1], F32, tag="ar0o")
            nc.scalar.dma_start(ar0_in[:], zt[:])
            nc.gpsimd.collective_compute(
                "AllReduce", mybir.AluOpType.add, replica_groups=RG,
                ins=[ar0_in.opt()], outs=[ar0_out.opt()])
            nc.scalar.dma_start(dbg_out, ar0_out[:])

            # ---- branch D inputs first: they gate the first matmuls ----
            ct_t = bg.tile([128, kd, BL], F32R, tag="ct")
            nc.sync.dma_start(ct_t[:], cdt)

            sq = cp.tile([128, BL], F32, tag="sq")

            # ================= branch D =================
            g_t = bg.tile([128, MD, BL], F32R, tag="g")
            for m in range(MD):
                xdb = ws.tile([128, kd, 128], F32R, tag="wblk")
                nc.sync.dma_start(xdb[:], xd[m])
                ps = pp.tile([128, BL], F32, tag="mm")
                for k in range(kd):
                    nc.tensor.matmul(ps[:], xdb[:, k, :], ct_t[:, k, :],
                                     start=(k == 0), stop=(k == kd - 1))
                nc.vector.tensor_copy(g_t[:, m, :], ps[:])

            # residents needed later: issued after the head DMAs so they
            # don't compete with the pipeline-critical first loads
            def vec_tile(src, nm, name):
                t = cp.tile([128, nm], F32, tag=name)
                nc.gpsimd.dma_start(t[:], src)
                return t

            b_dg_t = vec_tile(b_dg, MH, "b_dg")
            b_pg_t = vec_tile(b_pg, MH, "b_pg")
            g_e1_t = vec_tile(g_e1, M1, "g_e1")
            be_e1_t = vec_tile(be_e1, M1, "be_e1")
            g_e2_t = vec_tile(g_e2, M2, "g_e2")
            be_e2_t = vec_tile(be_e2, M2, "be_e2")
            b_o1_t = vec_tile(b_o1, M3, "b_o1")
            g_o_t = vec_tile(g_o, M3, "g_o")
            be_o_t = vec_tile(be_o, M3, "be_o")
            b_o2_t = cp.tile([1, 1], F32, tag="b_o2")
            nc.gpsimd.dma_start(b_o2_t[:], b_o2)

            # prefetch branch-P coefficient matrix into the same slot as cdt
            cpt_t = bg.tile([128, kp, BL], F32R, tag="ct")
            nc.gpsimd.dma_start(cpt_t[:], cpt)

            ft_t = bg.tile([128, KF, BL], F32R, tag="ft")
            nc.scalar.dma_start(ft_t[:, 0:11, :], fin)
            w_o1_t = cp.tile([128, M2, H3], F32R, tag="w_o1")
            nc.scalar.dma_start(w_o1_t[:], w_o1)
            w_o2_t = cp.tile([128, M3, 1], F32R, tag="w_o2")
            nc.scalar.dma_start(w_o2_t[:], w_o2)

            for m in range(MH):
                wdb = ws.tile([128, MD, 128], F32R, tag="wblk")
                nc.sync.dma_start(wdb[:], w_dg[m])
                ps = pp.tile([128, BL], F32, tag="mm")
                for k in range(MD):
                    nc.tensor.matmul(ps[:], wdb[:, k, :], g_t[:, k, :],
                                     start=(k == 0), stop=(k == MD - 1))
                nc.scalar.activation(ft_t[:, 11 + m, :], ps[:], AF.Lrelu,
                                     bias=b_dg_t[:, m:m + 1], alpha=0.01)

            # ================= branch P =================
            gp_t = bg.tile([128, MP, BL], F32R, tag="g")
            for m in range(MP):
                xpb = ws.tile([128, kp, 128], F32R, tag="wblk")
                nc.sync.dma_start(xpb[:], xp[m])
                ps = pp.tile([128, BL], F32, tag="mm")
                for k in range(kp):
                    nc.tensor.matmul(ps[:], xpb[:, k, :], cpt_t[:, k, :],
                                     start=(k == 0), stop=(k == kp - 1))
                nc.vector.tensor_copy(gp_t[:, m, :], ps[:])
            for m in range(MH):
                wpb = ws.tile([128, MP, 128], F32R, tag="wblk")
                nc.sync.dma_start(wpb[:], w_pg[m])
                ps = pp.tile([128, BL], F32, tag="mm")
                for k in range(MP):
                    nc.tensor.matmul(ps[:], wpb[:, k, :], gp_t[:, k, :],
                                     start=(k == 0), stop=(k == MP - 1))
                nc.scalar.activation(ft_t[:, 19 + m, :], ps[:], AF.Lrelu,
                                     bias=b_pg_t[:, m:m + 1], alpha=0.01)

            # ================= MLP =================
            def bn_scale_shift(stats_g, nm, g_t_, be_t_, goff, name):
                """stats_g [128, 2*nm] (sums 0:nm, sumsqs nm:2nm) ->
                scale/shift [128, nm] using gamma/beta columns goff:goff+nm."""
                mean = cp.tile([128, nm], F32, tag=f"{name}_mean")
                var = cp.tile([128, nm], F32, tag=f"{name}_var")
                scl = cp.tile([128, nm], F32, tag=f"{name}_scl")
                shf = cp.tile([128, nm], F32, tag=f"{name}_shf")
                nc.vector.tensor_scalar_mul(mean[:], stats_g[:, 0:nm], 1.0 / B)
                nc.vector.tensor_mul(var[:], mean[:], mean[:])
                nc.vector.tensor_scalar_mul(scl[:], stats_g[:, nm:2 * nm], 1.0 / B)
                nc.vector.tensor_sub(var[:], scl[:], var[:])
                nc.vector.tensor_scalar_add(var[:], var[:], EPS)
                nc.scalar.activation(var[:], var[:], AF.Sqrt)
                nc.vector.reciprocal(var[:], var[:])
                nc.vector.tensor_mul(scl[:], var[:], g_t_[:, goff:goff + nm])
                nc.vector.tensor_mul(shf[:], mean[:], scl[:])
                nc.vector.tensor_sub(shf[:], be_t_[:, goff:goff + nm], shf[:])
                return scl, shf

            def all_reduce_stats(stats, nm, name):
                ar_in = dp.tile([128, 2 * nm], F32, tag=f"{name}_i")
                ar_out = dp.tile([128, 2 * nm], F32, tag=f"{name}_o")
                nc.scalar.dma_start(ar_in[:], stats[:])
                nc.gpsimd.collective_compute(
                    "AllReduce", mybir.AluOpType.add, replica_groups=RG,
                    ins=[ar_in.opt()], outs=[ar_out.opt()])
                stats_g = cp.tile([128, 2 * nm], F32, tag=f"{name}_g")
                nc.scalar.dma_start(stats_g[:], ar_out[:])
                return stats_g

            # ---- layer e1 (3-way split stats to hide AR latency) ----
            SPLITS = [(0, 8), (8, 4), (12, 4)]
            stats1 = [cp.tile([128, 2 * n], F32, tag=f"stats1_{i}",
                              name=f"stats1_{i}")
                      for i, (_, n) in enumerate(SPLITS)]
            z1_t = bg.tile([128, M1, BL], F32R, tag="ct")
            stats_g1 = [None] * len(SPLITS)
            ends = {off + n - 1: i for i, (off, n) in enumerate(SPLITS)}
            for m in range(M1):
                wb = ws.tile([128, KF, 128], F32R, tag="wblk")
                nc.sync.dma_start(wb[:], w_e1[m])
                ps = pp.tile([128, BL], F32, tag="mm")
                for k in range(KF):
                    nc.tensor.matmul(ps[:], wb[:, k, :], ft_t[:, k, :],
                                     start=(k == 0), stop=(k == KF - 1))
                nc.vector.tensor_copy(z1_t[:, m, :], ps[:])
                si = next(i for i, (off, n) in enumerate(SPLITS)
                          if off <= m < off + n)
                off, n = SPLITS[si]
                mm_ = m - off
                nc.vector.reduce_sum(stats1[si][:, mm_:mm_ + 1], ps[:], axis=AX.X)
                nc.scalar.activation(sq[:], ps[:], AF.Square,
                                     accum_out=stats1[si][:, n + mm_:n + mm_ + 1])
                if m in ends:
                    i = ends[m]
                    stats_g1[i] = all_reduce_stats(stats1[i], SPLITS[i][1],
                                                   f"ar1_{i}")
            for i, (off, n) in enumerate(SPLITS):
                scl1, shf1 = bn_scale_shift(stats_g1[i], n, g_e1_t, be_e1_t,
                                            off, f"bn1{i}")
                for mm_ in range(n):
                    m = off + mm_
                    nc.scalar.activation(z1_t[:, m, :], z1_t[:, m, :], AF.Lrelu,
                                         bias=shf1[:, mm_:mm_ + 1],
                                         scale=scl1[:, mm_:mm_ + 1], alpha=0.01)

            # ---- layer e2 (two k-passes: first pass only needs h[0:K2A],
            #      which is ready while the second stats AllReduce flies) ----
            stats2 = cp.tile([128, 2 * M2], F32, tag="stats2")
            z2_t = bg.tile([128, M2, BL], F32R, tag="g")
            f_t = bg.tile([128, M2, BL], F32R, tag="ct")
            # pass-B weight slices live in the (now dead) ft slot
            wbb_t = bg.tile([128, M2, M1 - K2A, 128], F32R, tag="ft")
            nc.gpsimd.dma_start(wbb_t[:], w_e2b)
            for m in range(M2):
                wb = ws.tile([128, K2A, 128], F32R, tag="wblk")
                nc.sync.dma_start(wb[:], w_e2a[m])
                ps = pp.tile([128, BL], F32, tag="mm")
                for k in range(K2A):
                    nc.tensor.matmul(ps[:], wb[:, k, :], z1_t[:, k, :],
                                     start=(k == 0), stop=(k == K2A - 1))
                nc.vector.tensor_copy(z2_t[:, m, :], ps[:])
            for m in range(M2):
                ps = pp.tile([128, BL], F32, tag="mm")
                for k in range(K2A, M1):
                    nc.tensor.matmul(ps[:], wbb_t[:, m, k - K2A, :],
                                     z1_t[:, k, :],
                                     start=(k == K2A), stop=(k == M1 - 1))
                nc.vector.tensor_add(z2_t[:, m, :], z2_t[:, m, :], ps[:])
                nc.vector.reduce_sum(stats2[:, m:m + 1], z2_t[:, m, :], axis=AX.X)
                nc.scalar.activation(sq[:], z2_t[:, m, :], AF.Square,
                                     accum_out=stats2[:, M2 + m:M2 + m + 1])
            stats2_g = all_reduce_stats(stats2, M2, "ar2")
            scl2, shf2 = bn_scale_shift(stats2_g, M2, g_e2_t, be_e2_t, 0, "bn2")
            for m in range(M2):
                nc.scalar.activation(f_t[:, m, :], z2_t[:, m, :], AF.Lrelu,
                                     bias=shf2[:, m:m + 1],
                                     scale=scl2[:, m:m + 1], alpha=0.01)
            nc.scalar.dma_start(ft_out.rearrange("(m p) b -> p m b", p=128),
                                f_t[:])

            # ---- output head ----
            stats3 = cp.tile([128, 2 * M3], F32, tag="stats3")
            ot_t = bg.tile([128, M3, BL], F32, tag="g")
            for m in range(M3):
                ps = pp.tile([128, BL], F32, tag="mm")
                for k in range(M2):
                    nc.tensor.matmul(ps[:], w_o1_t[:, k, m * 128:(m + 1) * 128],
                                     f_t[:, k, :],
                                     start=(k == 0), stop=(k == M2 - 1))
                nc.scalar.activation(ot_t[:, m, :], ps[:], AF.Lrelu,
                                     bias=b_o1_t[:, m:m + 1], alpha=0.01)
                nc.vector.reduce_sum(stats3[:, m:m + 1], ot_t[:, m, :], axis=AX.X)
                nc.scalar.activation(sq[:], ot_t[:, m, :], AF.Square,
                                     accum_out=stats3[:, M3 + m:M3 + m + 1])
            stats3_g = all_reduce_stats(stats3, M3, "ar3")
            scl3, shf3 = bn_scale_shift(stats3_g, M3, g_o_t, be_o_t, 0, "bn3")
            on_t = ws.tile([128, M3, BL], F32R, tag="wblk")
            for m in range(M3):
                nc.scalar.activation(on_t[:, m, :], ot_t[:, m, :], AF.Identity,
                                     bias=shf3[:, m:m + 1],
                                     scale=scl3[:, m:m + 1])
            ps_y = pp.tile([128, BL], F32, tag="mm")
            for k in range(M3):
                nc.tensor.matmul(ps_y[0:1, :], w_o2_t[:, k, :], on_t[:, k, :],
                                 start=(k == 0), stop=(k == M3 - 1))
            yt_t = ws.tile([1, BL], F32, tag="wblk")
            nc.scalar.activation(yt_t[:], ps_y[0:1, :], AF.Identity,
                                 bias=b_o2_t[0:1, 0:1])
            nc.scalar.dma_start(yt_out, yt_t[:])

    nc.compile()
    return nc


def _get_program(rd_max, rp_max):
    key = (rd_max, rp_max)
    if key not in _PROGRAMS:
        _PROGRAMS[key] = _build_program(rd_max, rp_max)
    return _PROGRAMS[key]


def _block(w, kt, mt):
    """[kt*128, mt*128] -> [mt, 128, kt, 128] contiguous (lhsT tile layout)."""
    return np.ascontiguousarray(
        w.reshape(kt, 128, mt, 128).transpose(2, 1, 0, 3))


def _rows_block(x, kt, cols):
    """[kt*128, cols] -> [128, kt, cols] contiguous."""
    return np.ascontiguousarray(x.reshape(kt, 128, cols).transpose(1, 0, 2))


def _vec_block(v, mt):
    """[mt*128] -> [128, mt] contiguous."""
    return np.ascontiguousarray(np.asarray(v, np.float32).reshape(mt, 128).T)


def _prep_branch(index_full, x_nodes, edge_index, edge_w, n_nodes, r_max, k_pad):
    """Per-core gathered node features + transposed coefficient matrices,
    both in blocked device layout."""
    src = np.asarray(edge_index[0], dtype=np.int64)
    dst = np.asarray(edge_index[1], dtype=np.int64)
    w = np.asarray(edge_w, dtype=np.float32)
    index_full = np.asarray(index_full, dtype=np.int64)
    x_nodes = np.asarray(x_nodes, dtype=np.float32)
    kt, mt = r_max // 128, k_pad // 128

    deg = (np.bincount(dst, weights=w.astype(np.float64), minlength=n_nodes)
           .astype(np.float32) + np.float32(1.0))
    dis = np.float32(1.0) / np.sqrt(deg)
    norm = dis[src] * w * dis[dst]

    xgs, cts, rmax_seen = [], [], 0
    for c in range(N_CORES):
        idx = index_full[c * BL:(c + 1) * BL]
        uniq, inv = np.unique(idx, return_inverse=True)
        sel = np.isin(dst, uniq)
        es, ed, en = src[sel], dst[sel], norm[sel]
        cols = np.union1d(es, uniq)
        r = len(cols)
        rmax_seen = max(rmax_seen, r)
        if r > r_max:
            return None, None, rmax_seen
        cnt = np.zeros((r_max, len(uniq)), np.float32)
        np.add.at(cnt, (np.searchsorted(cols, es), np.searchsorted(uniq, ed)), en)
        cnt[np.searchsorted(cols, uniq), np.arange(len(uniq))] += dis[uniq] ** 2
        cts.append(_rows_block(cnt[:, inv], kt, BL))
        xg = np.zeros((r_max, k_pad), np.float32)
        xg[:r, :x_nodes.shape[1]] = x_nodes[cols]
        xgs.append(_block(xg, kt, mt))
    return xgs, cts, rmax_seen


def _round_up(x, q):
    return ((x + q - 1) // q) * q


def _prepare(inputs):
    """Host preprocessing: returns (compiled program, per-core in_maps)."""
    inp = {k: np.asarray(v) for k, v in inputs.items()}

    rd_max, rp_max = RD_MAX, RP_MAX
    while True:
        xd_l, cdt_l, seen_d = _prep_branch(
            inp["d_index"], inp["d_ecfps"], inp["d_edge_index"],
            inp["d_edge_weight"], ND, rd_max, DX)
        if xd_l is None:
            rd_max = _round_up(seen_d, 128)
            continue
        break
    while True:
        xp_l, cpt_l, seen_p = _prep_branch(
            inp["p_index"], inp["p_gos"], inp["p_edge_index"],
            inp["p_edge_weight"], NP, rp_max, PX_PAD)
        if xp_l is None:
            rp_max = _round_up(seen_p, 128)
            continue
        break

    f32 = np.float32
    w_pg = np.zeros((PX_PAD, H2), f32)
    w_pg[:PX] = inp["W_pg"]
    w_e1p = np.zeros((FDIM_PAD, H1), f32)
    w_e1p[0:300] = inp["W_e1"][0:300]
    w_e1p[384:1408] = inp["W_e1"][300:1324]
    w_e1p[1408:2432] = inp["W_e1"][1324:2348]
    w_e1p[2432:3456] = inp["W_e1"][2348:3372]

    common = {
        "w_dg": _block(np.asarray(inp["W_dg"], f32), MD, MH),
        "w_pg": _block(w_pg, MP, MH),
        "w_e1": _block(w_e1p, KF, M1),
        "w_e2a": np.ascontiguousarray(
            _block(np.asarray(inp["W_e2"], f32), M1, M2)[:, :, 0:K2A, :]),
        "w_e2b": np.ascontiguousarray(
            _block(np.asarray(inp["W_e2"], f32), M1, M2)[:, :, K2A:, :]
            .transpose(1, 0, 2, 3)),
        "w_o1": _rows_block(np.asarray(inp["W_o1"], f32), M2, H3),
        "w_o2": _rows_block(np.asarray(inp["W_o2"], f32), M3, 1),
        "b_dg": _vec_block(inp["b_dg"], MH),
        "b_pg": _vec_block(inp["b_pg"], MH),
        "g_e1": _vec_block(inp["g_e1"], M1),
        "be_e1": _vec_block(inp["be_e1"], M1),
        "g_e2": _vec_block(inp["g_e2"], M2),
        "be_e2": _vec_block(inp["be_e2"], M2),
        "b_o1": _vec_block(inp["b_o1"], M3),
        "g_o": _vec_block(inp["g_o"], M3),
        "be_o": _vec_block(inp["be_o"], M3),
        "b_o2": np.asarray(inp["b_o2"], f32).reshape(1, 1),
    }

    d_vecs = np.asarray(inp["d_vecs"], dtype=f32)
    p_emb = np.asarray(inp["p_embeddings"], dtype=f32)

    in_maps = []
    for c in range(N_CORES):
        rows = slice(c * BL, (c + 1) * BL)
        fin = np.zeros((FIN_ROWS, BL), f32)
        fin[0:300] = d_vecs[rows].T
        fin[384:1408] = p_emb[rows].T
        m = dict(common)
        m.update({
            "xd": xd_l[c], "cdt": cdt_l[c],
            "xp": xp_l[c], "cpt": cpt_l[c],
            "fin": _rows_block(fin, 11, BL),
        })
        in_maps.append(m)

    nc = _get_program(rd_max, rp_max)
    return nc, in_maps


def _assemble(results):
    f32 = np.float32
    y = np.empty((B, 1), f32)
    feature = np.empty((B, H2), f32)
    for c in range(N_CORES):
        y[c * BL:(c + 1) * BL, 0] = results[c]["yt_out"][0]
        feature[c * BL:(c + 1) * BL] = results[c]["ft_out"].T
    return y, feature


def kernel(**inputs):
    nc, in_maps = _prepare(inputs)
    res = run_bass_kernel_spmd(nc, in_maps, core_ids=list(range(N_CORES)))
    return _assemble(res.results)
